# revision 33
# baseline (speedup 1.0000x reference)
"""Trainium2 Bass kernel for nn_MemPIDModel (dense_cnn).

Strategy (8 NeuronCores, no collectives):
  - core c handles sample b = c//4 (trunk replicated within each 4-core group)
  - core c computes vocab shard v = c%4 of the tied head: [1024,512]@[512,8000]
  - trunk in "Layout A": activations kept as x^T [D=512 partitions (4 tiles), T=1024 free]
  - chunked wavefront: each layer processed in 2 chunks of 512 tokens, emitted as a
    software pipeline S2(c), S3(c), S1(c+2) so PE/DVE/GpSimd/ACT overlap across chunks
  - conv: dtile 0 on PE (15 diag-matmul accumulation), dtiles 1-3 as DVE STT chains
  - SwiGLU: per-nt psg/psu matmuls -> ACT silu/copy -> GpSimd pch mult -> psy matmuls
    with the residual base injected as a diag(gnorm) matmul; ACT copies PSUM->xA f32
  - norm scale: PE ones-matmul reduce -> ACT Abs_reciprocal_sqrt -> GpSimd broadcast
"""

import os
import sys
import numpy as np

sys.path.insert(0, "/opt/trn_rl_repo")

import ml_dtypes

B = 2
T = 1024
D = 512
HID = 1024
KK = 15
VOCAB = 32000
RANK = 64
NL = 6
MIX_W = 0.1
UP_DIL = [1, 2, 4, 8, 16, 32]
DN_DIL = UP_DIL[::-1]
EPS = 1e-6
NCORES = 8
VSHARDS = 4
VS = VOCAB // VSHARDS  # 8000
DT = D // 128  # 4 D-tiles
HT = HID // 128  # 8 H-tiles
TT2 = T // 512  # 2 T-chunks of 512
PAD = 448  # (K-1)*max_dil
CONVW = PAD + T  # padded conv input width per d-tile
NV = (VS + 511) // 512  # 16 head column chunks (last is 320 wide, zero-padded)

BF16 = ml_dtypes.bfloat16

# debug knobs (affect program shape; kernel cache key includes them)
N_LAYERS = int(os.environ.get("KB_LAYERS", str(NL)))
N_STACKS = int(os.environ.get("KB_STACKS", "3"))
SKIP_HEAD = bool(int(os.environ.get("KB_SKIP_HEAD", "0")))
DEBUG_TRUNK_OUT = bool(int(os.environ.get("KB_TRUNK_OUT", "0")))
# conv dtiles 0..PE_ND-1 run on PE as diag matmuls; the rest as DVE STT chains
PE_ND = int(os.environ.get("KB_PE_DTILES", "2"))

_prog_cache = {}


def _f32(x):
    return np.ascontiguousarray(np.asarray(x), dtype=np.float32)


def _bf(x):
    return np.ascontiguousarray(np.asarray(x).astype(np.float32), dtype=BF16)


def _cols(v):
    """[D] vector -> [128, DT] A-layout per-partition columns."""
    return np.ascontiguousarray(_f32(v).reshape(DT, 128).T)


def _stack_dils(stack_idx):
    return UP_DIL if stack_idx in (0, 2) else DN_DIL


def build_program():
    key = (N_LAYERS, N_STACKS, SKIP_HEAD, DEBUG_TRUNK_OUT, PE_ND)
    if key in _prog_cache:
        return _prog_cache[key]

    import concourse.bass as bass
    import concourse.mybir as mybir
    import concourse.tile as tile
    from concourse import bacc
    from concourse.masks import make_identity

    dt = mybir.dt
    Alu = mybir.AluOpType
    Act = mybir.ActivationFunctionType

    nc = bacc.Bacc(None, target_bir_lowering=False, debug=False)

    # ---------------- DRAM I/O ----------------
    d_idx = nc.dram_tensor("idx_rs", [128, 8], dt.int32, kind="ExternalInput")
    d_emb = nc.dram_tensor("emb_tbl", [VOCAB, D], dt.float32, kind="ExternalInput")
    d_pos = nc.dram_tensor("pos_rs", [128, 8 * D], dt.float32, kind="ExternalInput")
    d_cst = nc.dram_tensor("cst", [128, 64], dt.float32, kind="ExternalInput")
    d_rc = nc.dram_tensor("rc_bc", [128, T], dt.float32, kind="ExternalInput")
    d_mgw = nc.dram_tensor("mgwT", [D, D], dt.float32, kind="ExternalInput")

    d_cw = {}
    d_w13 = {}
    d_w2 = {}
    d_pid = {}
    d_cdiag = {}
    d_gdiag = {}
    for s in ("up", "dn"):
        d_cw[s] = nc.dram_tensor(
            f"{s}_cw", [NL, 128, DT * 16], dt.float32, kind="ExternalInput"
        )
        # w13p: [NL, 128, DT*2*HID] partition-major: row p col kt*2H+j = w13T[kt*128+p, j]
        d_w13[s] = nc.dram_tensor(
            f"{s}_w13p", [NL, 128, DT * 2 * HID], dt.bfloat16, kind="ExternalInput"
        )
        d_w2[s] = nc.dram_tensor(
            f"{s}_w2p", [NL, 128, HT * D], dt.bfloat16, kind="ExternalInput"
        )
        d_pid[s] = nc.dram_tensor(
            f"{s}_pid", [128, (NL - 1) * 12], dt.float32, kind="ExternalInput"
        )
        # conv diag tiles for PE dtiles: PE_ND x 15 taps x [128,128] diag
        d_cdiag[s] = nc.dram_tensor(
            f"{s}_cdiag", [NL, 128, PE_ND * KK * 128], dt.bfloat16, kind="ExternalInput"
        )
        # gnorm diag tiles (residual base inject): 4 dtiles x [128,128]
        d_gdiag[s] = nc.dram_tensor(
            f"{s}_gdiag", [NL, 128, DT * 128], dt.bfloat16, kind="ExternalInput"
        )
    d_dwT = nc.dram_tensor("sg_dwT", [3, D, RANK], dt.bfloat16, kind="ExternalInput")
    d_uwT = nc.dram_tensor("sg_uwT", [3, RANK, D], dt.bfloat16, kind="ExternalInput")
    d_sgc = nc.dram_tensor("sg_cols", [128, 16], dt.float32, kind="ExternalInput")
    # head emb shard: [NV, 128, DT*512]: [nv, p, kt*512+j] = embT[kt*128+p, nv*512+j]
    d_embT = nc.dram_tensor("embT_sh", [NV, 128, DT * 512], dt.bfloat16, kind="ExternalInput")

    d_out = nc.dram_tensor("logits_sh", [T, VS], dt.float32, kind="ExternalOutput")
    if DEBUG_TRUNK_OUT:
        d_trunk = nc.dram_tensor("trunk_out", [128, DT * T], dt.float32, kind="ExternalOutput")

    f32 = dt.float32
    bf = dt.bfloat16

    with tile.TileContext(nc) as tc:
        import contextlib

        ctx = contextlib.ExitStack()
        with ctx:
            const = ctx.enter_context(tc.tile_pool(name="const", bufs=1))
            master = ctx.enter_context(tc.tile_pool(name="master", bufs=1))
            lay = ctx.enter_context(tc.tile_pool(name="lay", bufs=1))
            wgt = ctx.enter_context(tc.tile_pool(name="wgt", bufs=2))
            psum = ctx.enter_context(tc.tile_pool(name="psum", bufs=1, space="PSUM"))

            # ---------------- constants ----------------
            epsc = const.tile([128, 1], f32, tag="epsc")
            nc.vector.memset(epsc[:], EPS)
            ones_bf = const.tile([128, 1], bf, tag="ones")
            nc.vector.memset(ones_bf[:], 1.0)
            ident = const.tile([128, 128], f32, tag="ident")
            make_identity(nc, ident[:])
            cst = const.tile([128, 64], f32, tag="cst")
            nc.sync.dma_start(cst[:], d_cst[:])
            rc_bc = const.tile([128, T], f32, tag="rc")
            nc.sync.dma_start(rc_bc[:], d_rc[:])
            sgc = const.tile([128, 16], f32, tag="sgc")
            nc.sync.dma_start(sgc[:], d_sgc[:])
            pidc = {}
            for s in ("up", "dn"):
                pidc[s] = const.tile(
                    [128, (NL - 1) * 12], f32, tag=f"pid_{s}", name=f"pid_{s}"
                )
                nc.sync.dma_start(pidc[s][:], d_pid[s][:])

            # persistent activations (A-layout, free index = dt*T + t)
            xA = master.tile([128, DT * T], f32, tag="xA")

            def keep_tile():  # initial, then gated2 (sequential lifetimes)
                return master.tile([128, DT * T], f32, tag="keep", name="keep")

            def f32a_tile():  # integ during stacks / mixed during boundaries
                return lay.tile([128, DT * T], f32, tag="f32a", name="f32a")

            # per-chunk work tiles [128, DT*512]
            def t_zb():
                return lay.tile([128, DT * 512], bf, tag="zb", name="zb", bufs=1)

            def t_ub():
                return lay.tile([128, DT * 512], bf, tag="ub", name="ub", bufs=2)

            def t_sq():
                return lay.tile([128, DT * 512], bf, tag="sq", name="sq", bufs=1)

            def t_hb():
                return lay.tile([128, DT * 512], bf, tag="hb", name="hb", bufs=2)

            def t_cacc():
                return lay.tile([128, max(DT - PE_ND, 1) * 512], bf, tag="cacc", name="cacc", bufs=1)

            def t_sbc():
                return lay.tile([128, 512], bf, tag="sbc", name="sbc", bufs=1)

            def t_srow():
                return lay.tile([1, 512], bf, tag="srow", name="srow", bufs=1)

            def t_pch():
                return lay.tile([128, HT * 512], bf, tag="pch", name="pch", bufs=1)

            def t_gst():
                return lay.tile([128, 512], bf, tag="gst", name="gst", bufs=2)

            def t_ust():
                return lay.tile([128, 512], bf, tag="ust", name="ust", bufs=2)

            def t_xnb():
                return lay.tile([128, DT * CONVW], bf, tag="xnb", name="xnb", bufs=2)

            def ps_tile(tag):
                return psum.tile([128, 512], f32, tag=tag, bufs=2, name=tag)

            # ---------------- P0: gather + embnorm + shift + mem ----------------
            with tc.tile_pool(name="p0", bufs=1) as p0:
                idx_sb = p0.tile([128, 8], dt.int32, tag="idx")
                nc.sync.dma_start(idx_sb[:], d_idx[:])
                # borrow the f32a (integ) buffer: same shape/dtype, first real
                # use is after p0
                gth = lay.tile([128, DT * T], f32, tag="f32a", name="gth")
                for c in range(8):
                    nc.gpsimd.indirect_dma_start(
                        out=gth[:, c * D : (c + 1) * D],
                        out_offset=None,
                        in_=d_emb[:],
                        in_offset=bass.IndirectOffsetOnAxis(ap=idx_sb[:, c : c + 1], axis=0),
                    )
                for c in range(4):
                    pos_sb = p0.tile([128, 2 * D], f32, tag="pos", bufs=2)
                    nc.sync.dma_start(pos_sb[:], d_pos[:, c * 2 * D : (c + 1) * 2 * D])
                    nc.vector.tensor_tensor(
                        out=gth[:, c * 2 * D : (c + 1) * 2 * D],
                        in0=gth[:, c * 2 * D : (c + 1) * 2 * D],
                        in1=pos_sb[:],
                        op=Alu.add,
                    )
                ss = p0.tile([128, 8], f32, tag="ss")
                sqt = p0.tile([128, D], f32, tag="sqt")
                for c in range(8):
                    nc.scalar.activation(
                        sqt[:],
                        gth[:, c * D : (c + 1) * D],
                        Act.Square,
                        accum_out=ss[:, c : c + 1],
                    )
                nc.scalar.activation(ss[:], ss[:], Act.Ln, bias=epsc[:], scale=1.0 / D)
                nc.scalar.activation(ss[:], ss[:], Act.Exp, scale=-0.5)
                for c in range(8):
                    nc.vector.tensor_scalar(
                        gth[:, c * D : (c + 1) * D],
                        gth[:, c * D : (c + 1) * D],
                        ss[:, c : c + 1],
                        None,
                        Alu.mult,
                    )
                # transpose B->A (borrow the keep buffer: initial is copied after p0)
                x_n = master.tile([128, DT * T], f32, tag="keep", name="xn_a")
                for c in range(8):
                    pst = ps_tile("ps_g")
                    for dtt in range(DT):
                        nc.tensor.transpose(
                            out=pst[:, dtt * 128 : (dtt + 1) * 128],
                            in_=gth[:, c * D + dtt * 128 : c * D + (dtt + 1) * 128],
                            identity=ident[:],
                        )
                    for dtt in range(DT):
                        nc.vector.tensor_copy(
                            x_n[:, dtt * T + c * 128 : dtt * T + (c + 1) * 128],
                            pst[:, dtt * 128 : (dtt + 1) * 128],
                        )
                # mem gate
                ps_mem = ps_tile("ps_u")
                for kt in range(DT):
                    mgw_sb = p0.tile([128, D], f32, tag="mgw", name="mgw_sb", bufs=1)
                    nc.sync.dma_start(mgw_sb[:], d_mgw[kt * 128 : (kt + 1) * 128, :])
                    for m in range(DT):
                        nc.tensor.matmul(
                            ps_mem[:, m : m + 1],
                            lhsT=mgw_sb[:, m * 128 : (m + 1) * 128],
                            rhs=cst[:, 16 + kt : 17 + kt],
                            start=(kt == 0),
                            stop=(kt == DT - 1),
                        )
                tmem = p0.tile([128, 4], f32, tag="tmem")
                for m in range(DT):
                    nc.scalar.activation(
                        tmem[:, m : m + 1],
                        ps_mem[:, m : m + 1],
                        Act.Tanh,
                        scale=0.5,
                        bias=cst[:, 12 + m : 13 + m],
                    )
                nc.vector.tensor_scalar(tmem[:], tmem[:], 0.5, 0.5, Alu.mult, Alu.add)
                # token shift + mem
                tsh = p0.tile([128, T], f32, tag="pos", name="tsh", bufs=2)
                for dtt in range(DT):
                    o = dtt * T
                    nc.vector.tensor_scalar(
                        xA[:, o : o + 1], x_n[:, o : o + 1], cst[:, dtt : dtt + 1], None, Alu.mult
                    )
                    nc.vector.tensor_scalar(
                        tsh[:, 1:T],
                        x_n[:, o + 1 : o + T],
                        cst[:, 8 + dtt : 9 + dtt],
                        None,
                        Alu.mult,
                    )
                    nc.vector.scalar_tensor_tensor(
                        out=xA[:, o + 1 : o + T],
                        in0=x_n[:, o : o + T - 1],
                        scalar=cst[:, 4 + dtt : 5 + dtt],
                        in1=tsh[:, 1:T],
                        op0=Alu.mult,
                        op1=Alu.add,
                    )
                    nc.vector.tensor_scalar(
                        xA[:, o : o + T], xA[:, o : o + T], tmem[:, dtt : dtt + 1], None, Alu.add
                    )
            initial = keep_tile()
            nc.vector.tensor_scalar(initial[:], xA[:], 1.0, None, Alu.mult)

            # ---------------- conv block stack (chunked wavefront) ----------------
            def load_layer_weights(s, li):
                w = {}
                w["cw"] = wgt.tile([128, DT * 16], f32, tag="cw", name="cw")
                nc.sync.dma_start(w["cw"][:], d_cw[s][li])
                w["w13"] = wgt.tile([128, DT * 2 * HID], bf, tag="w13", name="w13")
                for q in range(8):
                    qs = q * HID
                    nc.sync.dma_start(
                        w["w13"][:, qs : qs + HID], d_w13[s][li, :, qs : qs + HID]
                    )
                w["w2"] = wgt.tile([128, HT * D], bf, tag="w2", name="w2")
                for q in range(2):
                    qs = q * 4 * D
                    nc.gpsimd.dma_start(
                        w["w2"][:, qs : qs + 4 * D], d_w2[s][li, :, qs : qs + 4 * D]
                    )
                w["cdiag"] = wgt.tile([128, PE_ND * KK * 128], bf, tag="cdiag", name="cdiag")
                nc.gpsimd.dma_start(w["cdiag"][:], d_cdiag[s][li])
                w["gdiag"] = wgt.tile([128, DT * 128], bf, tag="gdiag", name="gdiag")
                nc.gpsimd.dma_start(w["gdiag"][:], d_gdiag[s][li])
                return w

            def run_stack(stack_idx):
                s = "up" if stack_idx in (0, 2) else "dn"
                dils = _stack_dils(stack_idx)
                integ = f32a_tile()
                nc.vector.tensor_scalar(integ[:], xA[:], 1.0, None, Alu.mult)

                wts = {}  # li -> weight tiles
                st = {}  # chunk c -> dict of tiles
                for li in range(min(2, N_LAYERS)):
                    wts[li] = load_layer_weights(s, li)

                def S1dve(c):
                    # PID gate: ACT ki-scale + DVE STT -> zb (li>0 only)
                    li, nt = c // 2, c % 2
                    o5 = nt * 512
                    cs = st[c] = {}
                    if li > 0:
                        pc = pidc[s]
                        pb = (li - 1) * 12
                        zb = cs["zb"] = t_zb()
                        for dtt in range(DT):
                            nc.vector.tensor_scalar(
                                zb[:, dtt * 512 : (dtt + 1) * 512],
                                xA[:, dtt * T + o5 : dtt * T + o5 + 512],
                                pc[:, pb + dtt : pb + 1 + dtt],
                                None,
                                Alu.mult,
                            )
                            nc.vector.scalar_tensor_tensor(
                                out=zb[:, dtt * 512 : (dtt + 1) * 512],
                                in0=integ[:, dtt * T + o5 : dtt * T + o5 + 512],
                                scalar=pc[:, pb + 4 + dtt : pb + 5 + dtt],
                                in1=zb[:, dtt * 512 : (dtt + 1) * 512],
                                op0=Alu.mult,
                                op1=Alu.add,
                            )

                def S1rest(c):
                    # silu -> norm scale -> normed conv input
                    li, nt = c // 2, c % 2
                    d = dils[li]
                    o5 = nt * 512
                    cs = st[c]
                    ub = cs["ub"] = t_ub()
                    if li > 0:
                        nc.scalar.activation(ub[:], cs["zb"][:], Act.Silu)
                    else:
                        for dtt in range(DT):
                            nc.scalar.activation(
                                ub[:, dtt * 512 : (dtt + 1) * 512],
                                xA[:, dtt * T + o5 : dtt * T + o5 + 512],
                                Act.Copy,
                            )
                    sq = cs["sq"] = t_sq()
                    nc.vector.tensor_tensor(out=sq[:], in0=ub[:], in1=ub[:], op=Alu.mult)
                    if nt == 0:
                        xnb = st[("xnb", li)] = t_xnb()
                        for dtt in range(DT):
                            ob = dtt * CONVW
                            nc.gpsimd.memset(xnb[:, ob + PAD - 14 * d : ob + PAD], 0.0)

                def S1norm(c):
                    # norm reduce (PE) + rsqrt (ACT) + bcast (GP) + xnb (DVE)
                    li, nt = c // 2, c % 2
                    o5 = nt * 512
                    cs = st[c]
                    ub = cs["ub"]
                    sq = cs["sq"]
                    xnb = st[("xnb", li)]
                    ps = ps_tile("ps_cn")
                    for kt in range(DT):
                        nc.tensor.matmul(
                            ps[0:1, :],
                            lhsT=ones_bf[:],
                            rhs=sq[:, kt * 512 : (kt + 1) * 512],
                            start=(kt == 0),
                            stop=(kt == DT - 1),
                        )
                    srow = t_srow()
                    nc.scalar.activation(
                        srow[:], ps[0:1, :], Act.Abs_reciprocal_sqrt,
                        bias=epsc[0:1, :], scale=1.0 / D,
                    )
                    sbc = t_sbc()
                    nc.gpsimd.partition_broadcast(sbc[:], srow[0:1, :])
                    for dtt in range(DT):
                        nc.vector.tensor_tensor(
                            out=xnb[:, dtt * CONVW + PAD + o5 : dtt * CONVW + PAD + o5 + 512],
                            in0=ub[:, dtt * 512 : (dtt + 1) * 512],
                            in1=sbc[:],
                            op=Alu.mult,
                        )

                def S2pe(c):
                    # conv dtiles 0..PE_ND-1: 15 diag-matmul accumulation on PE
                    li, nt = c // 2, c % 2
                    d = dils[li]
                    o5 = nt * 512
                    cs = st[c]
                    w = wts[li]
                    xnb = st[("xnb", li)]
                    hb = cs["hb"]
                    for dtt in range(PE_ND):
                        psc = ps_tile("ps_cn")
                        for m in range(KK - 1, -1, -1):
                            stt = dtt * CONVW + PAD + o5 - m * d
                            nc.tensor.matmul(
                                psc[:],
                                lhsT=w["cdiag"][:, (dtt * KK + m) * 128 : (dtt * KK + m + 1) * 128],
                                rhs=xnb[:, stt : stt + 512],
                                start=(m == KK - 1),
                                stop=(m == 0),
                            )
                        nc.scalar.activation(
                            hb[:, dtt * 512 : (dtt + 1) * 512],
                            psc[:],
                            Act.Silu,
                            bias=w["cw"][:, dtt * 16 + 15 : dtt * 16 + 16],
                        )

                def S2dve(c):
                    # conv dtiles 1-3: DVE STT chains + ACT silu into hb
                    li, nt = c // 2, c % 2
                    d = dils[li]
                    o5 = nt * 512
                    cs = st[c]
                    w = wts[li]
                    xnb = st[("xnb", li)]
                    hb = cs["hb"] = t_hb()
                    cacc = t_cacc()
                    for dtt in range(PE_ND, DT):
                        ob = dtt * CONVW
                        oc = (dtt - PE_ND) * 512
                        wb_ = dtt * 16

                        def tap_in(m):
                            stt = ob + PAD + o5 - m * d
                            return xnb[:, stt : stt + 512]

                        nc.vector.tensor_scalar(
                            cacc[:, oc : oc + 512], tap_in(14),
                            w["cw"][:, wb_ + 14 : wb_ + 15], None, Alu.mult,
                        )
                        for m in range(13, -1, -1):
                            nc.vector.scalar_tensor_tensor(
                                out=cacc[:, oc : oc + 512],
                                in0=tap_in(m),
                                scalar=w["cw"][:, wb_ + m : wb_ + m + 1],
                                in1=cacc[:, oc : oc + 512],
                                op0=Alu.mult,
                                op1=Alu.add,
                            )
                        nc.scalar.activation(
                            hb[:, dtt * 512 : (dtt + 1) * 512],
                            cacc[:, oc : oc + 512],
                            Act.Silu,
                            bias=w["cw"][:, wb_ + 15 : wb_ + 16],
                        )

                def S3(c):
                    li, nt = c // 2, c % 2
                    o5 = nt * 512
                    cs = st[c]
                    w = wts[li]
                    xnb = st[("xnb", li)]
                    hb = cs["hb"]
                    # phase A: g/u + pch
                    pch = t_pch()
                    for kh in range(HT):
                        psg = ps_tile("ps_g")
                        for kt in range(DT):
                            nc.tensor.matmul(
                                psg[:],
                                lhsT=w["w13"][:, kt * 2 * HID + kh * 128 : kt * 2 * HID + (kh + 1) * 128],
                                rhs=hb[:, kt * 512 : (kt + 1) * 512],
                                start=(kt == 0),
                                stop=(kt == DT - 1),
                            )
                        gst = t_gst()
                        nc.scalar.activation(gst[:], psg[:], Act.Silu)
                        psu = ps_tile("ps_u")
                        for kt in range(DT):
                            nc.tensor.matmul(
                                psu[:],
                                lhsT=w["w13"][:, kt * 2 * HID + HID + kh * 128 : kt * 2 * HID + HID + (kh + 1) * 128],
                                rhs=hb[:, kt * 512 : (kt + 1) * 512],
                                start=(kt == 0),
                                stop=(kt == DT - 1),
                            )
                        ust = t_ust()
                        nc.scalar.activation(ust[:], psu[:], Act.Copy)
                        eng = nc.gpsimd if kh % 2 == 0 else nc.vector
                        eng.tensor_tensor(
                            out=pch[:, kh * 512 : (kh + 1) * 512],
                            in0=gst[:],
                            in1=ust[:],
                            op=Alu.mult,
                        )
                        if kh == 2 and c + 1 < 2 * N_LAYERS:
                            S1norm(c + 1)
                    # phase B: psy with residual base injected as diag matmul
                    for md in range(DT):
                        psy = ps_tile("ps_y")
                        if li > 0:
                            base = xnb[:, md * CONVW + PAD + o5 : md * CONVW + PAD + o5 + 512]
                        else:
                            # cur_in for layer 0 is raw xA; ub holds its bf16 cast
                            base = cs["ub"][:, md * 512 : (md + 1) * 512]
                        nc.tensor.matmul(
                            psy[:],
                            lhsT=w["gdiag"][:, md * 128 : (md + 1) * 128],
                            rhs=base,
                            start=True,
                            stop=False,
                        )
                        for kh in range(HT):
                            nc.tensor.matmul(
                                psy[:],
                                lhsT=w["w2"][:, kh * D + md * 128 : kh * D + (md + 1) * 128],
                                rhs=pch[:, kh * 512 : (kh + 1) * 512],
                                start=False,
                                stop=(kh == HT - 1),
                            )
                        xs = xA[:, md * T + o5 : md * T + o5 + 512]
                        nc.scalar.activation(xs, psy[:], Act.Copy)
                    if li < N_LAYERS - 1:
                        for dtt in range(DT):
                            nc.gpsimd.tensor_tensor(
                                out=integ[:, dtt * T + o5 : dtt * T + o5 + 512],
                                in0=integ[:, dtt * T + o5 : dtt * T + o5 + 512],
                                in1=xA[:, dtt * T + o5 : dtt * T + o5 + 512],
                                op=Alu.add,
                            )
                    # weight prefetch for layer li+2
                    if nt == 1 and li + 2 < N_LAYERS:
                        wts[li + 2] = load_layer_weights(s, li + 2)

                # software pipeline over chunks: while PE runs swiglu(c), DVE
                # runs conv(c+1); S1(c+1) pieces are interleaved so no engine
                # FIFO head-of-line blocks another chunk's work.
                NCH = 2 * N_LAYERS
                S1dve(0)
                S1rest(0)
                S1norm(0)
                S2dve(0)
                for c in range(NCH):
                    if c + 1 < NCH:
                        S1dve(c + 1)
                    S2pe(c)
                    if c + 1 < NCH:
                        S1rest(c + 1)
                    S3(c)  # also emits S1norm(c+1) mid-phase-A
                    if c + 1 < NCH:
                        S2dve(c + 1)
                    st.pop(c, None)

            # ---------------- mix + sgate boundary ----------------
            def boundary(k, old_tile):
                mixed = f32a_tile()  # integ dead
                # causal cumsum: chunked scan + carry
                cs01 = t_ub()  # [128, 2048] scratch: dtiles 0,1 full-T
                cs23 = t_ub()
                cstl = [(cs01, 0), (cs01, 1), (cs23, 0), (cs23, 1)]
                for dtt in range(DT):
                    buf, half = cstl[dtt]
                    o = half * 1024
                    for ntc in range(TT2):
                        nc.vector.tensor_tensor_scan(
                            out=buf[:, o + ntc * 512 : o + (ntc + 1) * 512],
                            data0=xA[:, dtt * T + ntc * 512 : dtt * T + (ntc + 1) * 512],
                            data1=xA[:, dtt * T + ntc * 512 : dtt * T + (ntc + 1) * 512],
                            initial=0.0,
                            op0=Alu.add,
                            op1=Alu.bypass,
                        )
                    carry32 = lay.tile([128, 1], f32, tag="carry", name="carry", bufs=2)
                    nc.scalar.activation(carry32[:], buf[:, o + 511 : o + 512], Act.Copy)
                    nc.vector.tensor_scalar(
                        buf[:, o + 512 : o + 1024],
                        buf[:, o + 512 : o + 1024],
                        carry32[:],
                        None,
                        Alu.add,
                    )
                    # mixed = xA + cs * rc
                    nc.gpsimd.tensor_tensor(
                        out=buf[:, o : o + 1024],
                        in0=buf[:, o : o + 1024],
                        in1=rc_bc[:],
                        op=Alu.mult,
                    )
                    nc.vector.tensor_tensor(
                        out=mixed[:, dtt * T : (dtt + 1) * T],
                        in0=xA[:, dtt * T : (dtt + 1) * T],
                        in1=buf[:, o : o + 1024],
                        op=Alu.add,
                    )

                dw_sb = wgt.tile([128, DT * RANK], bf, tag="dw", name="dw")
                for kt in range(DT):
                    nc.sync.dma_start(
                        dw_sb[:, kt * RANK : (kt + 1) * RANK],
                        d_dwT[k, kt * 128 : (kt + 1) * 128, :],
                    )
                uw_sb = wgt.tile([128, D], bf, tag="uw", name="uw")
                nc.sync.dma_start(uw_sb[0:RANK, :], d_uwT[k])

                # rmsnorm(mixed) chunked; nb consumed by sgate matmuls
                hsb = lay.tile([128, T], bf, tag="hsb", name="hsb")
                tgf = [t_hb(), t_hb()]
                for ntc in range(TT2):
                    o5 = ntc * 512
                    ubx = t_zb()
                    for dtt in range(DT):
                        nc.scalar.activation(
                            ubx[:, dtt * 512 : (dtt + 1) * 512],
                            mixed[:, dtt * T + o5 : dtt * T + o5 + 512],
                            Act.Copy,
                        )
                    sq = t_sq()
                    nc.vector.tensor_tensor(out=sq[:], in0=ubx[:], in1=ubx[:], op=Alu.mult)
                    ps = ps_tile("ps_cn")
                    for kt in range(DT):
                        nc.tensor.matmul(
                            ps[0:1, :],
                            lhsT=ones_bf[:],
                            rhs=sq[:, kt * 512 : (kt + 1) * 512],
                            start=(kt == 0),
                            stop=(kt == DT - 1),
                        )
                    srow = t_srow()
                    nc.scalar.activation(
                        srow[:], ps[0:1, :], Act.Abs_reciprocal_sqrt,
                        bias=epsc[0:1, :], scale=1.0 / D,
                    )
                    sbc = t_sbc()
                    nc.gpsimd.partition_broadcast(sbc[:], srow[0:1, :])
                    nb = t_ub()
                    for dtt in range(DT):
                        nc.gpsimd.tensor_tensor(
                            out=nb[:, dtt * 512 : (dtt + 1) * 512],
                            in0=ubx[:, dtt * 512 : (dtt + 1) * 512],
                            in1=sbc[:],
                            op=Alu.mult,
                        )
                    psh = ps_tile("ps_g")
                    for kt in range(DT):
                        nc.tensor.matmul(
                            psh[0:RANK, :],
                            lhsT=dw_sb[:, kt * RANK : (kt + 1) * RANK],
                            rhs=nb[:, kt * 512 : (kt + 1) * 512],
                            start=(kt == 0),
                            stop=(kt == DT - 1),
                        )
                    nc.scalar.activation(
                        hsb[0:RANK, o5 : o5 + 512],
                        psh[0:RANK, :],
                        Act.Silu,
                        bias=sgc[0:RANK, k : k + 1],
                    )
                    for md in range(DT):
                        psg2 = ps_tile("ps_u")
                        nc.tensor.matmul(
                            psg2[:],
                            lhsT=uw_sb[0:RANK, md * 128 : (md + 1) * 128],
                            rhs=hsb[0:RANK, o5 : o5 + 512],
                            start=True,
                            stop=True,
                        )
                        nc.scalar.activation(
                            tgf[ntc][:, md * 512 : (md + 1) * 512],
                            psg2[:],
                            Act.Tanh,
                            scale=0.5,
                            bias=sgc[:, 4 + k * 4 + md : 5 + k * 4 + md],
                        )
                # blend: xA = old + (0.5 + 0.5*t) * (mixed - old)
                for ntc in range(TT2):
                    o5 = ntc * 512
                    for dtt in range(DT):
                        xs = xA[:, dtt * T + o5 : dtt * T + o5 + 512]
                        tgs = tgf[ntc][:, dtt * 512 : (dtt + 1) * 512]
                        dfs = t_gst()
                        nc.vector.tensor_tensor(
                            out=dfs[:],
                            in0=mixed[:, dtt * T + o5 : dtt * T + o5 + 512],
                            in1=old_tile[:, dtt * T + o5 : dtt * T + o5 + 512],
                            op=Alu.subtract,
                        )
                        nc.vector.tensor_scalar(tgs, tgs, 0.5, 0.5, Alu.mult, Alu.add)
                        nc.gpsimd.tensor_tensor(out=dfs[:], in0=tgs, in1=dfs[:], op=Alu.mult)
                        nc.vector.tensor_tensor(
                            out=xs,
                            in0=old_tile[:, dtt * T + o5 : dtt * T + o5 + 512],
                            in1=dfs[:],
                            op=Alu.add,
                        )

            # ---------------- run the model ----------------
            gated2 = None
            for si in range(N_STACKS):
                run_stack([0, 1, 2][si])
                if si == 0:
                    boundary(0, initial)
                elif si == 1:
                    boundary(1, initial)
                    gated2 = keep_tile()  # initial dead
                    nc.vector.tensor_scalar(gated2[:], xA[:], 1.0, None, Alu.mult)
                elif si == 2:
                    boundary(2, gated2)

            if DEBUG_TRUNK_OUT:
                nc.sync.dma_start(d_trunk[:], xA[:])

            # ---------------- final rmsnorm + tied head ----------------
            if not SKIP_HEAD:
                hd = ctx.enter_context(tc.tile_pool(name="hd", bufs=1))
                ob = hd.tile([128, DT * T], bf, tag="obf", name="obf")
                s_colT = hd.tile([128, 8], f32, tag="scolT", name="scolT")
                for ntc in range(TT2):
                    o5 = ntc * 512
                    for dtt in range(DT):
                        nc.scalar.activation(
                            ob[:, dtt * T + o5 : dtt * T + o5 + 512],
                            xA[:, dtt * T + o5 : dtt * T + o5 + 512],
                            Act.Copy,
                        )
                    sq = t_sq()
                    for dtt in range(DT):
                        nc.vector.tensor_tensor(
                            out=sq[:, dtt * 512 : (dtt + 1) * 512],
                            in0=ob[:, dtt * T + o5 : dtt * T + o5 + 512],
                            in1=ob[:, dtt * T + o5 : dtt * T + o5 + 512],
                            op=Alu.mult,
                        )
                    ps = ps_tile("ps_cn")
                    for kt in range(DT):
                        nc.tensor.matmul(
                            ps[0:1, :],
                            lhsT=ones_bf[:],
                            rhs=sq[:, kt * 512 : (kt + 1) * 512],
                            start=(kt == 0),
                            stop=(kt == DT - 1),
                        )
                    srf = hd.tile([1, 512], f32, tag="srf", name="srf", bufs=1)
                    nc.scalar.activation(
                        srf[0:1, :], ps[0:1, :], Act.Abs_reciprocal_sqrt,
                        bias=epsc[0:1, :], scale=1.0 / D,
                    )
                    pst = ps_tile("ps_g")
                    for c in range(4):
                        nc.tensor.transpose(
                            out=pst[:, c * 128 : c * 128 + 1],
                            in_=srf[0:1, c * 128 : (c + 1) * 128],
                            identity=ident[0:1, 0:1],
                        )
                    for c in range(4):
                        nc.vector.tensor_copy(
                            s_colT[:, ntc * 4 + c : ntc * 4 + c + 1],
                            pst[:, c * 128 : c * 128 + 1],
                        )
                for nv in range(NV):
                    nw = min(512, VS - nv * 512)
                    rhsb = hd.tile([128, DT * 512], bf, tag="rhsb", name="rhsb", bufs=2)
                    for q in range(4):
                        nc.sync.dma_start(
                            rhsb[:, q * 512 : (q + 1) * 512], d_embT[nv, :, q * 512 : (q + 1) * 512]
                        )
                    for mt in range(8):
                        psl = ps_tile(["ps_g", "ps_u", "ps_y", "ps_cn"][mt % 4])
                        for kt in range(DT):
                            nc.tensor.matmul(
                                psl[:, :nw],
                                lhsT=ob[:, kt * T + mt * 128 : kt * T + (mt + 1) * 128],
                                rhs=rhsb[:, kt * 512 : kt * 512 + nw],
                                start=(kt == 0),
                                stop=(kt == DT - 1),
                            )
                        lsb = hd.tile([128, 512], f32, tag="lsb", name="lsb", bufs=1)
                        if mt % 2 == 0:
                            nc.scalar.activation(
                                lsb[:, :nw], psl[:, :nw], Act.Copy, scale=s_colT[:, mt : mt + 1]
                            )
                        else:
                            nc.vector.tensor_scalar(
                                lsb[:, :nw], psl[:, :nw], s_colT[:, mt : mt + 1], None, Alu.mult
                            )
                        nc.gpsimd.dma_start(
                            d_out[mt * 128 : (mt + 1) * 128, nv * 512 : nv * 512 + nw],
                            lsb[:, :nw],
                        )

    nc.finalize()
    _prog_cache[key] = nc
    return nc


def prep_inputs(inputs):
    """Host-side: full model inputs -> list of 8 per-core in_maps."""
    idx = np.asarray(inputs["idx"])
    emb = _f32(inputs["emb"])
    pos = _f32(inputs["pos"])[0, :T]  # [T, D]
    we = _f32(inputs["emb_norm_w"])
    ts = _f32(inputs["token_shift"])
    mgw = _f32(inputs["mem_gate_w"])
    mgb = _f32(inputs["mem_gate_b"])
    memp = _f32(inputs["memory_p"])
    fnw = _f32(inputs["final_norm_w"])

    pos_rs = np.ascontiguousarray(
        pos.reshape(8, 128, D).transpose(1, 0, 2).reshape(128, 8 * D)
    )
    cst = np.zeros((128, 64), np.float32)
    cst[:, 0:4] = _cols(we)
    cst[:, 4:8] = _cols(ts * we)
    cst[:, 8:12] = _cols((1.0 - ts) * we)
    cst[:, 12:16] = _cols(0.5 * mgb)
    rc = (MIX_W / np.arange(1, T + 1, dtype=np.float32))[None, :]
    rc_bc = np.ascontiguousarray(np.broadcast_to(rc, (128, T)))
    mgwT = np.ascontiguousarray(mgw.T)

    stack_in = {}
    for s in ("up", "dn"):
        nw = _f32(inputs[f"{s}_norm_w"])  # [NL, D]
        cw = _f32(inputs[f"{s}_conv_w"])[:, :, 0, :]  # [NL, D, K]
        cb = _f32(inputs[f"{s}_conv_b"])  # [NL, D]
        w1 = _f32(inputs[f"{s}_w1"])
        w2 = _f32(inputs[f"{s}_w2"])
        w3 = _f32(inputs[f"{s}_w3"])
        kp = _f32(inputs[f"{s}_kp"])
        ki = _f32(inputs[f"{s}_ki"])
        gn = _f32(inputs[f"{s}_gnorm"])
        cwp = np.zeros((NL, 128, DT * 16), np.float32)
        cdiag = np.zeros((NL, 128, PE_ND * KK * 128), np.float32)
        gdiag = np.zeros((NL, 128, DT * 128), np.float32)
        for li in range(NL):
            # conv path: gnorm cancels inside the double rmsnorm; fold norm_w only.
            # gnorm survives only in the residual base (gdiag).
            gfold = gn[li - 1] if li > 0 else np.ones(D, np.float32)
            cwf = cw[li] * nw[li][:, None]  # [D, K]
            taps = cwf[:, ::-1]  # tap m multiplies shift m*d
            cwp[li, :, :] = np.concatenate(
                [
                    np.concatenate(
                        [taps.reshape(DT, 128, KK)[dtt], cb[li].reshape(DT, 128)[dtt][:, None]],
                        axis=1,
                    )
                    for dtt in range(DT)
                ],
                axis=1,
            )
            # conv diag tiles for PE dtiles
            for dtt in range(PE_ND):
                for m in range(KK):
                    cdiag[li, :, (dtt * KK + m) * 128 : (dtt * KK + m + 1) * 128] = np.diag(
                        taps[dtt * 128 : (dtt + 1) * 128, m]
                    )
            # gnorm diag (residual base inject, li>0 only; row li holds gn[li-1])
            for dtt in range(DT):
                gdiag[li, :, dtt * 128 : (dtt + 1) * 128] = np.diag(
                    gfold[dtt * 128 : (dtt + 1) * 128]
                )
        pid = np.zeros((128, (NL - 1) * 12), np.float32)
        for li in range(1, NL):
            pb = (li - 1) * 12
            pid[:, pb : pb + 4] = _cols(kp[li - 1])
            pid[:, pb + 4 : pb + 8] = _cols(ki[li - 1] / li)
            pid[:, pb + 8 : pb + 12] = _cols(gn[li - 1])
        stack_in[f"{s}_cw"] = np.ascontiguousarray(cwp)
        stack_in[f"{s}_pid"] = pid
        stack_in[f"{s}_cdiag"] = np.ascontiguousarray(cdiag).astype(BF16)
        stack_in[f"{s}_gdiag"] = np.ascontiguousarray(gdiag).astype(BF16)
        # partition-major packed weights: [NL, 128, DT*2*HID] and [NL, 128, HT*D]
        w13 = np.concatenate([w1.transpose(0, 2, 1), w3.transpose(0, 2, 1)], axis=2)
        # w13[li] is [D, 2*HID]; row p col kt*2H+j = w13[li][kt*128+p, j]
        w13p = np.ascontiguousarray(
            w13.reshape(NL, DT, 128, 2 * HID).transpose(0, 2, 1, 3).reshape(NL, 128, DT * 2 * HID)
        )
        stack_in[f"{s}_w13p"] = w13p.astype(BF16)
        w2T = w2.transpose(0, 2, 1)  # [NL, HID, D]
        w2p = np.ascontiguousarray(
            w2T.reshape(NL, HT, 128, D).transpose(0, 2, 1, 3).reshape(NL, 128, HT * D)
        )
        stack_in[f"{s}_w2p"] = w2p.astype(BF16)

    sgn = _f32(inputs["sg_norm"])
    sgdw = _f32(inputs["sg_down_w"])
    sgdb = _f32(inputs["sg_down_b"])
    sguw = _f32(inputs["sg_up_w"])
    sgub = _f32(inputs["sg_up_b"])
    dwT = np.stack(
        [np.ascontiguousarray(sgdw[k].T * sgn[k][:, None]) for k in range(3)]
    ).astype(BF16)
    uwT = np.stack([np.ascontiguousarray(sguw[k].T) for k in range(3)]).astype(BF16)
    sgc = np.zeros((128, 16), np.float32)
    for k in range(3):
        sgc[0:RANK, k] = sgdb[k]
        sgc[:, 4 + k * 4 : 8 + k * 4] = _cols(0.5 * sgub[k])

    embT = np.ascontiguousarray((emb.T * fnw[:, None]))  # [D, V] f32

    common = dict(
        pos_rs=pos_rs,
        cst=None,  # per-core (p_col differs)
        rc_bc=rc_bc,
        mgwT=mgwT,
        emb_tbl=emb,
        sg_dwT=dwT,
        sg_uwT=uwT,
        sg_cols=sgc,
        **stack_in,
    )

    in_maps = []
    for c in range(NCORES):
        b = c // 4
        vsh = c % 4
        m = dict(common)
        cstc = cst.copy()
        cstc[:, 16:20] = _cols(memp[b])
        m["cst"] = cstc
        m["idx_rs"] = np.ascontiguousarray(
            idx[b].astype(np.int32).reshape(8, 128).T
        )
        esh = embT[:, vsh * VS : (vsh + 1) * VS]  # [D, VS]
        eshpad = np.zeros((D, NV * 512), np.float32)
        eshpad[:, :VS] = esh
        # [NV, 128, DT*512]: [nv, p, kt*512+j] = eshpad[kt*128+p, nv*512+j]
        eshp = np.ascontiguousarray(
            eshpad.reshape(DT, 128, NV, 512).transpose(2, 1, 0, 3).reshape(NV, 128, DT * 512)
        )
        m["embT_sh"] = eshp.astype(BF16)
        in_maps.append(m)
    return in_maps


LAST_RESULTS = None


def kernel(**inputs):
    global LAST_RESULTS
    from concourse.bass_utils import run_bass_kernel_spmd

    nc = build_program()
    in_maps = prep_inputs(inputs)
    trace = bool(int(os.environ.get("KB_TRACE", "0")))
    res = run_bass_kernel_spmd(nc, in_maps, core_ids=list(range(NCORES)), trace=trace)
    LAST_RESULTS = res
    out = np.zeros((B, T, VOCAB), np.float32)
    for c in range(NCORES):
        b = c // 4
        vsh = c % 4
        out[b, :, vsh * VS : (vsh + 1) * VS] = res.results[c]["logits_sh"]
    return out
